# revision 1
# baseline (speedup 1.0000x reference)
"""Grouped-expert SwiGLU (MoE) Bass kernel for 8 TRN2 NeuronCores.

Problem: tokens pre-sorted by expert with per-expert counts; for expert e's
token slice xs: y = (silu(xs @ G_e^T) * (xs @ U_e^T)) @ D_e^T.

Strategy (all host logic; device program is uniform SPMD across 8 cores):
  * Tokens are split into 512-token blocks (counts are multiples of 512).
  * The 32 blocks are decomposed into 8 pieces of 3 blocks + 8 pieces of
    1 block, each piece single-expert; every core gets one 3-piece and one
    1-piece => exactly 2048 tokens/core, perfectly balanced compute.
  * Per piece the core receives that expert's weights (pre-transposed on
    host). Inside, gate/up/down matmuls run in float32r (TF32-like PE mode:
    full fp32 bits in memory, ~1 cyc/row on the PE, ~2e-4 relative error).
  * Output y is written in [token, dim] layout per core and scattered back
    on the host.
"""

import numpy as np

import concourse.tile as tile
from concourse import bacc, mybir
from concourse.bass_utils import run_bass_kernel_spmd

TB = 512  # token block
NCORES = 8

_PROGRAM_CACHE = {}


# --------------------------------------------------------------------------
# device program
# --------------------------------------------------------------------------
def build_program(piece_sizes, dim, hid, reps=1):
    """Uniform per-core program: for each piece i of piece_sizes[i] blocks,
    compute SwiGLU of its tokens with weight set i.

    Inputs:  xt [dim, T] (f32r, token-major columns), per piece: g{i},u{i}
    [dim,hid], d{i} [hid,dim] (f32r).  Output: y [T, dim] f32.

    reps > 1 re-executes the whole computation (for timing by differencing).
    """
    key = (tuple(piece_sizes), dim, hid, reps)
    if key in _PROGRAM_CACHE:
        return _PROGRAM_CACHE[key]

    f32 = mybir.dt.float32
    f32r = mybir.dt.float32r
    KD = dim // 128  # k-tiles for gate/up contraction
    HB = hid // 128  # h-tiles
    NCH = dim // 512  # output dim chunks
    T = sum(piece_sizes) * TB

    nc = bacc.Bacc("TRN2", target_bir_lowering=False, debug=False, num_devices=NCORES)
    xt = nc.dram_tensor("xt", [dim, T], f32r, kind="ExternalInput").ap()
    gs, us, ds = [], [], []
    for i in range(len(piece_sizes)):
        gs.append(nc.dram_tensor(f"g{i}", [dim, hid], f32r, kind="ExternalInput").ap())
        us.append(nc.dram_tensor(f"u{i}", [dim, hid], f32r, kind="ExternalInput").ap())
        ds.append(nc.dram_tensor(f"d{i}", [hid, dim], f32r, kind="ExternalInput").ap())
    # output is transposed: y[dim, T] (host transposes back)
    y = nc.dram_tensor("y", [dim, T], f32, kind="ExternalOutput").ap()

    max_sz = max(piece_sizes)
    with tile.TileContext(nc) as tc:
        with (
            tc.tile_pool(name="xp", bufs=max_sz) as xp,
            tc.tile_pool(name="h1p", bufs=1) as h1p,
            tc.tile_pool(name="wp", bufs=2) as wp,
            tc.tile_pool(name="dwp", bufs=2) as dwp,
            tc.tile_pool(name="actp", bufs=3) as actp,
            tc.tile_pool(name="otp", bufs=3) as otp,
            tc.tile_pool(name="psgu", bufs=3, space="PSUM") as psgu,
            tc.tile_pool(name="psop", bufs=2, space="PSUM") as psop,
        ):
          for _rep in range(reps):
            t0 = 0  # token offset (in tokens)
            for pi, sz in enumerate(piece_sizes):
                Tp = sz * TB
                h1 = h1p.tile([128, HB, Tp], f32r, tag="h1")
                # ---- phase 1: h1[h, t] = silu(G^T x) * (U^T x).
                # x blocks are separate tiles (pool slots) so each block's
                # matmuls start as soon as its own DMA lands; gate/up weights
                # stream once per piece (hb-outer loop).
                xws = []
                for tb in range(sz):
                    xw = xp.tile([128, KD, TB], f32r, tag="x")
                    nc.sync.dma_start(
                        out=xw,
                        in_=xt[:, t0 + tb * TB : t0 + (tb + 1) * TB].rearrange(
                            "(kd p) t -> p kd t", p=128
                        ),
                    )
                    xws.append(xw)
                for hb in range(HB):
                    gw = wp.tile([128, KD, 128], f32r, tag="gw")
                    uw = wp.tile([128, KD, 128], f32r, tag="uw")
                    nc.scalar.dma_start(
                        out=gw,
                        in_=gs[pi][:, hb * 128 : (hb + 1) * 128].rearrange(
                            "(kd p) h -> p kd h", p=128
                        ),
                    )
                    nc.scalar.dma_start(
                        out=uw,
                        in_=us[pi][:, hb * 128 : (hb + 1) * 128].rearrange(
                            "(kd p) h -> p kd h", p=128
                        ),
                    )
                    for tb in range(sz):
                        xw = xws[tb]
                        psg = psgu.tile([128, TB], f32, tag="psg")
                        psu = psgu.tile([128, TB], f32, tag="psu")
                        for kd in range(KD):
                            nc.tensor.matmul(
                                psg,
                                gw[:, kd, :],
                                xw[:, kd, :],
                                start=(kd == 0),
                                stop=(kd == KD - 1),
                            )
                        for kd in range(KD):
                            nc.tensor.matmul(
                                psu,
                                uw[:, kd, :],
                                xw[:, kd, :],
                                start=(kd == 0),
                                stop=(kd == KD - 1),
                            )
                        act = actp.tile([128, TB], f32, tag="act")
                        nc.scalar.activation(
                            act, psg, mybir.ActivationFunctionType.Silu
                        )
                        nc.vector.tensor_mul(
                            h1[:, hb, tb * TB : (tb + 1) * TB], act, psu
                        )
                # ---- phase 2: yT[c, t] = sum_h D^T[h, c] * h1[h, t]
                # stationary = 128x128 D^T column tiles (tiny SBUF footprint),
                # moving = h1 (already resident)
                for dcb in range(dim // 128):
                    dw = dwp.tile([128, HB, 128], f32r, tag="dw")
                    nc.scalar.dma_start(
                        out=dw,
                        in_=ds[pi][:, dcb * 128 : (dcb + 1) * 128].rearrange(
                            "(hb p) c -> p hb c", p=128
                        ),
                    )
                    for tcol in range(Tp // 512):
                        pso = psop.tile([128, 512], f32, tag="pso")
                        for hb in range(HB):
                            nc.tensor.matmul(
                                pso,
                                dw[:, hb, :],
                                h1[:, hb, tcol * 512 : (tcol + 1) * 512],
                                start=(hb == 0),
                                stop=(hb == HB - 1),
                            )
                        ot = otp.tile([128, 512], f32, tag="ot")
                        nc.vector.tensor_copy(ot, pso)
                        nc.sync.dma_start(
                            out=y[
                                dcb * 128 : (dcb + 1) * 128,
                                t0 + tcol * 512 : t0 + (tcol + 1) * 512,
                            ],
                            in_=ot,
                        )
                t0 += Tp
    nc.compile()
    _PROGRAM_CACHE[key] = nc
    return nc


# --------------------------------------------------------------------------
# host-side planning
# --------------------------------------------------------------------------
def plan_pieces(block_counts):
    """Decompose per-expert block counts into 8 cores x uniform piece sizes.

    Returns (piece_sizes, plans) where plans[core] = [(expert, block_start,
    nblocks), ...] with block_start in global padded block coordinates.
    Tries the (1, 3) split (balanced, min weight traffic); falls back to
    single-block pieces.
    """
    E = len(block_counts)
    starts = np.zeros(E, dtype=np.int64)
    np.cumsum(block_counts[:-1], out=starts[1:])
    total = int(np.sum(block_counts))

    if total == 4 * NCORES:
        # try k3[e] three-pieces + k1[e] singles with sum(k3) == 8
        k3 = [int(c) // 3 for c in block_counts]
        while sum(k3) > NCORES:
            e = max(range(E), key=lambda i: k3[i])
            k3[e] -= 1
        if sum(k3) == NCORES:
            threes, ones = [], []
            for e in range(E):
                b = int(block_counts[e])
                s = int(starts[e])
                for _ in range(k3[e]):
                    threes.append((e, s, 3))
                    s += 3
                while s < int(starts[e]) + b:
                    ones.append((e, s, 1))
                    s += 1
            assert len(threes) == NCORES and len(ones) == NCORES
            # pair same-expert pieces on the same core where possible;
            # big piece first so the small piece's input loads hide under
            # the big piece's down-projection compute
            plans = []
            used1 = [False] * NCORES
            for t in threes:
                j = next(
                    (
                        i
                        for i in range(NCORES)
                        if not used1[i] and ones[i][0] == t[0]
                    ),
                    None,
                )
                if j is None:
                    j = next(i for i in range(NCORES) if not used1[i])
                used1[j] = True
                plans.append([t, ones[j]])
            return (3, 1), plans

    # fallback: single-block pieces, padded to a multiple of NCORES with
    # dummy zero blocks (expert 0 weights, output discarded)
    per_core = -(-total // NCORES)
    pieces = []
    for e in range(E):
        for b in range(int(block_counts[e])):
            pieces.append((e, int(starts[e]) + b, 1))
    while len(pieces) < per_core * NCORES:
        pieces.append((0, -1, 1))  # dummy
    plans = [pieces[c * per_core : (c + 1) * per_core] for c in range(NCORES)]
    return tuple([1] * per_core), plans


def prepare(x, gate_proj, up_proj, down_proj, num_tokens_per_expert):
    """Host-side planning + per-core input construction.

    Returns (piece_sizes, plans, in_maps, scatter_info).
    """
    x = np.ascontiguousarray(np.asarray(x, dtype=np.float32))
    gate_proj = np.asarray(gate_proj, dtype=np.float32)
    up_proj = np.asarray(up_proj, dtype=np.float32)
    down_proj = np.asarray(down_proj, dtype=np.float32)
    counts = np.asarray(num_tokens_per_expert).astype(np.int64)

    T, dim = x.shape
    E, hid, _ = gate_proj.shape

    # ---- pad each expert's token segment to a multiple of TB (no-op for the
    # staged problem where every count is already a multiple of 512)
    offs = np.concatenate([[0], np.cumsum(counts)])
    pad_counts = ((counts + TB - 1) // TB) * TB
    if np.array_equal(pad_counts, counts):
        x_pad = x
        pad_offs = offs
        padded = False
    else:
        pad_offs = np.concatenate([[0], np.cumsum(pad_counts)])
        x_pad = np.zeros((int(pad_offs[-1]), dim), dtype=np.float32)
        for e in range(E):
            x_pad[pad_offs[e] : pad_offs[e] + counts[e]] = x[offs[e] : offs[e + 1]]
        padded = True

    block_counts = pad_counts // TB
    piece_sizes, plans = plan_pieces(block_counts)

    # ---- per-expert transposed weights
    GT = [np.ascontiguousarray(gate_proj[e].T) for e in range(E)]
    UT = [np.ascontiguousarray(up_proj[e].T) for e in range(E)]
    DT = [np.ascontiguousarray(down_proj[e].T) for e in range(E)]

    in_maps = []
    for c in range(NCORES):
        plan = plans[c]
        xs = []
        for (e, bs, nb) in plan:
            if bs < 0:
                xs.append(np.zeros((nb * TB, dim), dtype=np.float32))
            else:
                xs.append(x_pad[bs * TB : (bs + nb) * TB])
        xc = np.concatenate(xs, axis=0) if len(xs) > 1 else xs[0]
        m = {"xt": np.ascontiguousarray(xc.T)}
        for i, (e, bs, nb) in enumerate(plan):
            m[f"g{i}"] = GT[e]
            m[f"u{i}"] = UT[e]
            m[f"d{i}"] = DT[e]
        in_maps.append(m)

    scatter_info = (T, dim, E, offs, pad_offs, counts, padded)
    return piece_sizes, plans, in_maps, scatter_info


def scatter(per_core_y, plans, scatter_info):
    """Assemble the full output from per-core y arrays."""
    T, dim, E, offs, pad_offs, counts, padded = scatter_info
    out_pad = np.empty((int(pad_offs[-1]), dim), dtype=np.float32)
    for c in range(NCORES):
        yc = per_core_y[c]  # [dim, T_core] (transposed on device)
        t = 0
        for (e, bs, nb) in plans[c]:
            if bs >= 0:
                out_pad[bs * TB : (bs + nb) * TB] = yc[:, t : t + nb * TB].T
            t += nb * TB
    if not padded:
        return out_pad
    out = np.empty((T, dim), dtype=np.float32)
    for e in range(E):
        out[offs[e] : offs[e + 1]] = out_pad[pad_offs[e] : pad_offs[e] + counts[e]]
    return out


def kernel(x, gate_proj, up_proj, down_proj, num_tokens_per_expert):
    piece_sizes, plans, in_maps, scatter_info = prepare(
        x, gate_proj, up_proj, down_proj, num_tokens_per_expert
    )
    dim = scatter_info[1]
    hid = np.asarray(gate_proj).shape[1]
    nc = build_program(piece_sizes, dim, hid)
    res = run_bass_kernel_spmd(nc, in_maps, core_ids=list(range(NCORES)))
    return scatter([res.results[c]["y"] for c in range(NCORES)], plans, scatter_info)



# revision 3
# speedup vs baseline: 1.3012x; 1.3012x over previous
"""Grouped-expert SwiGLU (MoE) Bass kernel for 8 TRN2 NeuronCores.

Problem: tokens pre-sorted by expert with per-expert counts; for expert e's
token slice xs: y = (silu(xs @ G_e^T) * (xs @ U_e^T)) @ D_e^T.

Strategy (all host logic; device program is uniform SPMD across 8 cores):
  * Tokens are split into 512-token blocks (counts are multiples of 512).
  * The 32 blocks are decomposed into 8 pieces of 3 blocks + 8 pieces of
    1 block, each piece single-expert; every core gets one 3-piece and one
    1-piece => exactly 2048 tokens/core, perfectly balanced compute.
  * All operands are cast to bf16 on host (PE rate identical to f32r, but
    half the HBM traffic / SBUF footprint; abs accuracy ~3e-3 rel, well
    inside the 2e-2 gate). PSUM accumulation stays f32.
  * Every DMA source is pre-packed on host into the exact [partition,
    free] contiguous layout the device needs, so DMA descriptors are
    2-16 KB/partition contiguous runs (2x packet efficiency vs the
    strided 512B gathers of the naive layout).
  * Weight streams are issued from the scalar HWDGE queue (gate/up) and
    sync HWDGE queue (down + x + y) with enough pool buffers for one-
    iteration-ahead prefetch, keeping the PE matmul stream stall-free
    and the HAM clock-gate warm.
  * Output y is written bf16 in [dim-chunk, 128, T] layout per core and
    scattered/upcast back on the host.
"""

import numpy as np
import ml_dtypes

import concourse.tile as tile
from concourse import bacc, mybir
from concourse.bass_utils import run_bass_kernel_spmd

BF16 = ml_dtypes.bfloat16
TB = 512  # token block
NCORES = 8

_PROGRAM_CACHE = {}


# --------------------------------------------------------------------------
# device program
# --------------------------------------------------------------------------
def build_program(piece_sizes, dim, hid, reps=1):
    """Uniform per-core program: for each piece i of piece_sizes[i] blocks,
    compute SwiGLU of its tokens with weight set i.

    Inputs (all bf16, host-prepacked so each DMA is contiguous/partition):
      xb        [nblk, 128, KD*TB]   x block b: [p][kd][t] = x[b*TB+t, kd*128+p]
      g{i},u{i} [HB, 128, KD*128]    [hb][p][kd][h] = W[hb*128+h, kd*128+p]
      d{i}      [NCH, 128, HB*128]   [cb][p][hb][c] = D[cb*128+c, hb*128+p]
    Output:
      y         [NCH, 128, T]  bf16  [cb][p][t] = out[t, cb*128+p]
    """
    key = (tuple(piece_sizes), dim, hid, reps)
    if key in _PROGRAM_CACHE:
        return _PROGRAM_CACHE[key]

    f32 = mybir.dt.float32
    bf16 = mybir.dt.bfloat16
    KD = dim // 128   # k-tiles for gate/up contraction
    HB = hid // 128   # h-tiles
    NCH = dim // 128  # output dim chunks
    nblk = sum(piece_sizes)
    T = nblk * TB

    nc = bacc.Bacc("TRN2", target_bir_lowering=False, debug=False, num_devices=NCORES)
    xb = nc.dram_tensor("xb", [nblk, 128, KD * TB], bf16, kind="ExternalInput").ap()
    gs, us, ds = [], [], []
    for i in range(len(piece_sizes)):
        gs.append(nc.dram_tensor(f"g{i}", [HB, 128, KD * 128], bf16, kind="ExternalInput").ap())
        us.append(nc.dram_tensor(f"u{i}", [HB, 128, KD * 128], bf16, kind="ExternalInput").ap())
        ds.append(nc.dram_tensor(f"d{i}", [NCH, 128, HB * 128], bf16, kind="ExternalInput").ap())
    y = nc.dram_tensor("y", [NCH, 128, T], bf16, kind="ExternalOutput").ap()

    max_sz = max(piece_sizes)
    with tile.TileContext(nc) as tc:
        with (
            tc.tile_pool(name="xp", bufs=min(nblk, max_sz + 1)) as xp,
            tc.tile_pool(name="h1p", bufs=min(2, len(piece_sizes))) as h1p,
            tc.tile_pool(name="wp", bufs=2) as wp,
            tc.tile_pool(name="dwp", bufs=3) as dwp,
            tc.tile_pool(name="actp", bufs=3) as actp,
            tc.tile_pool(name="otp", bufs=3) as otp,
            # pool slots are per-tag: psgu holds psg+psu tags (2 bufs each =
            # 4 banks), psop 3 banks -> 7 of 8 PSUM banks
            tc.tile_pool(name="psgu", bufs=2, space="PSUM") as psgu,
            tc.tile_pool(name="psop", bufs=3, space="PSUM") as psop,
        ):
          for _rep in range(reps):
            blk0 = 0  # global block index
            for pi, sz in enumerate(piece_sizes):
                Tp = sz * TB
                h1 = h1p.tile([128, HB, Tp], bf16, tag="h1")
                # ---- phase 1: h1[h, t] = silu(G^T x) * (U^T x).
                # x blocks are separate tiles; each block's DMA is split in
                # two halves so the first matmuls start after ~half a block
                # has landed. gate/up weights stream once per piece
                # (hb-outer loop) from the scalar HWDGE queue.
                xws = []
                for tb in range(sz):
                    xw = xp.tile([128, KD, TB], bf16, tag="x")
                    half = KD // 2
                    nc.sync.dma_start(
                        out=xw[:, :half, :],
                        in_=xb[blk0 + tb, :, : half * TB].rearrange(
                            "p (kd t) -> p kd t", t=TB
                        ),
                    )
                    nc.sync.dma_start(
                        out=xw[:, half:, :],
                        in_=xb[blk0 + tb, :, half * TB :].rearrange(
                            "p (kd t) -> p kd t", t=TB
                        ),
                    )
                    xws.append(xw)
                for hb in range(HB):
                    gw = wp.tile([128, KD, 128], bf16, tag="gw")
                    uw = wp.tile([128, KD, 128], bf16, tag="uw")
                    nc.scalar.dma_start(
                        out=gw,
                        in_=gs[pi][hb].rearrange("p (kd h) -> p kd h", h=128),
                    )
                    nc.scalar.dma_start(
                        out=uw,
                        in_=us[pi][hb].rearrange("p (kd h) -> p kd h", h=128),
                    )
                    for tb in range(sz):
                        xw = xws[tb]
                        psg = psgu.tile([128, TB], f32, tag="psg")
                        psu = psgu.tile([128, TB], f32, tag="psu")
                        for kd in range(KD):
                            nc.tensor.matmul(
                                psg,
                                gw[:, kd, :],
                                xw[:, kd, :],
                                start=(kd == 0),
                                stop=(kd == KD - 1),
                            )
                        for kd in range(KD):
                            nc.tensor.matmul(
                                psu,
                                uw[:, kd, :],
                                xw[:, kd, :],
                                start=(kd == 0),
                                stop=(kd == KD - 1),
                            )
                        act = actp.tile([128, TB], f32, tag="act")
                        nc.scalar.activation(
                            act, psg, mybir.ActivationFunctionType.Silu
                        )
                        nc.vector.tensor_mul(
                            h1[:, hb, tb * TB : (tb + 1) * TB], act, psu
                        )
                # ---- phase 2: yT[c, t] = sum_h D^T[h, c] * h1[h, t]
                # stationary = 128x128 D^T column tiles, moving = h1
                # (already resident). dw DMAs ride the sync queue so their
                # kickoffs run ahead of the compute stream.
                for dcb in range(NCH):
                    dw = dwp.tile([128, HB, 128], bf16, tag="dw")
                    nc.sync.dma_start(
                        out=dw,
                        in_=ds[pi][dcb].rearrange("p (hb c) -> p hb c", c=128),
                    )
                    for tcol in range(Tp // 512):
                        pso = psop.tile([128, 512], f32, tag="pso")
                        for hb in range(HB):
                            nc.tensor.matmul(
                                pso,
                                dw[:, hb, :],
                                h1[:, hb, tcol * 512 : (tcol + 1) * 512],
                                start=(hb == 0),
                                stop=(hb == HB - 1),
                            )
                        ot = otp.tile([128, 512], bf16, tag="ot")
                        nc.vector.tensor_copy(ot, pso)
                        nc.sync.dma_start(
                            out=y[
                                dcb,
                                :,
                                blk0 * TB + tcol * 512 : blk0 * TB + (tcol + 1) * 512,
                            ],
                            in_=ot,
                        )
                blk0 += sz
    nc.move_matmul_waits_to_ldweights()
    nc.compile()
    _PROGRAM_CACHE[key] = nc
    return nc


# --------------------------------------------------------------------------
# host-side planning
# --------------------------------------------------------------------------
def plan_pieces(block_counts):
    """Decompose per-expert block counts into 8 cores x uniform piece sizes.

    Returns (piece_sizes, plans) where plans[core] = [(expert, block_start,
    nblocks), ...] with block_start in global padded block coordinates.
    Tries the (3, 1) split (balanced, min weight traffic); falls back to
    single-block pieces.
    """
    E = len(block_counts)
    starts = np.zeros(E, dtype=np.int64)
    np.cumsum(block_counts[:-1], out=starts[1:])
    total = int(np.sum(block_counts))

    if total == 4 * NCORES:
        # try k3[e] three-pieces + k1[e] singles with sum(k3) == 8
        k3 = [int(c) // 3 for c in block_counts]
        while sum(k3) > NCORES:
            e = max(range(E), key=lambda i: k3[i])
            k3[e] -= 1
        if sum(k3) == NCORES:
            threes, ones = [], []
            for e in range(E):
                b = int(block_counts[e])
                s = int(starts[e])
                for _ in range(k3[e]):
                    threes.append((e, s, 3))
                    s += 3
                while s < int(starts[e]) + b:
                    ones.append((e, s, 1))
                    s += 1
            assert len(threes) == NCORES and len(ones) == NCORES
            # pair same-expert pieces on the same core where possible;
            # big piece first so the small piece's input loads hide under
            # the big piece's down-projection compute
            plans = []
            used1 = [False] * NCORES
            for t in threes:
                j = next(
                    (
                        i
                        for i in range(NCORES)
                        if not used1[i] and ones[i][0] == t[0]
                    ),
                    None,
                )
                if j is None:
                    j = next(i for i in range(NCORES) if not used1[i])
                used1[j] = True
                plans.append([t, ones[j]])
            return (3, 1), plans

    # fallback: single-block pieces, padded to a multiple of NCORES with
    # dummy zero blocks (expert 0 weights, output discarded)
    per_core = -(-total // NCORES)
    pieces = []
    for e in range(E):
        for b in range(int(block_counts[e])):
            pieces.append((e, int(starts[e]) + b, 1))
    while len(pieces) < per_core * NCORES:
        pieces.append((0, -1, 1))  # dummy
    plans = [pieces[c * per_core : (c + 1) * per_core] for c in range(NCORES)]
    return tuple([1] * per_core), plans


def _pack_gu(w, KD, HB):
    """[hid, dim] f32 -> [HB, 128, KD*128] bf16 with
    out[hb, p, kd*128+h] = w[hb*128+h, kd*128+p]."""
    hid, dim = w.shape
    arr = w.reshape(HB, 128, KD, 128).transpose(0, 3, 2, 1)  # hb, p, kd, h
    return np.ascontiguousarray(arr.astype(BF16).reshape(HB, 128, KD * 128))


def _pack_d(w, NCH, HB):
    """[dim, hid] f32 -> [NCH, 128, HB*128] bf16 with
    out[cb, p, hb*128+c] = w[cb*128+c, hb*128+p]."""
    dim, hid = w.shape
    arr = w.reshape(NCH, 128, HB, 128).transpose(0, 3, 2, 1)  # cb, p, hb, c
    return np.ascontiguousarray(arr.astype(BF16).reshape(NCH, 128, HB * 128))


def _pack_x(xc, KD):
    """[T, dim] f32 -> [nblk, 128, KD*TB] bf16 with
    out[b, p, kd*TB+t] = x[b*TB+t, kd*128+p]."""
    T, dim = xc.shape
    nblk = T // TB
    arr = xc.reshape(nblk, TB, KD, 128).transpose(0, 3, 2, 1)  # b, p, kd, t
    return np.ascontiguousarray(arr.astype(BF16).reshape(nblk, 128, KD * TB))


def prepare(x, gate_proj, up_proj, down_proj, num_tokens_per_expert):
    """Host-side planning + per-core input construction.

    Returns (piece_sizes, plans, in_maps, scatter_info).
    """
    x = np.ascontiguousarray(np.asarray(x, dtype=np.float32))
    gate_proj = np.asarray(gate_proj, dtype=np.float32)
    up_proj = np.asarray(up_proj, dtype=np.float32)
    down_proj = np.asarray(down_proj, dtype=np.float32)
    counts = np.asarray(num_tokens_per_expert).astype(np.int64)

    T, dim = x.shape
    E, hid, _ = gate_proj.shape
    KD, HB, NCH = dim // 128, hid // 128, dim // 128

    # ---- pad each expert's token segment to a multiple of TB (no-op for the
    # staged problem where every count is already a multiple of 512)
    offs = np.concatenate([[0], np.cumsum(counts)])
    pad_counts = ((counts + TB - 1) // TB) * TB
    if np.array_equal(pad_counts, counts):
        x_pad = x
        pad_offs = offs
        padded = False
    else:
        pad_offs = np.concatenate([[0], np.cumsum(pad_counts)])
        x_pad = np.zeros((int(pad_offs[-1]), dim), dtype=np.float32)
        for e in range(E):
            x_pad[pad_offs[e] : pad_offs[e] + counts[e]] = x[offs[e] : offs[e + 1]]
        padded = True

    block_counts = pad_counts // TB
    piece_sizes, plans = plan_pieces(block_counts)

    # ---- per-expert packed bf16 weights (shared across cores)
    GP = [_pack_gu(gate_proj[e], KD, HB) for e in range(E)]
    UP = [_pack_gu(up_proj[e], KD, HB) for e in range(E)]
    DP = [_pack_d(down_proj[e], NCH, HB) for e in range(E)]

    in_maps = []
    for c in range(NCORES):
        plan = plans[c]
        xs = []
        for (e, bs, nb) in plan:
            if bs < 0:
                xs.append(np.zeros((nb * TB, dim), dtype=np.float32))
            else:
                xs.append(x_pad[bs * TB : (bs + nb) * TB])
        xc = np.concatenate(xs, axis=0) if len(xs) > 1 else xs[0]
        m = {"xb": _pack_x(xc, KD)}
        for i, (e, bs, nb) in enumerate(plan):
            m[f"g{i}"] = GP[e]
            m[f"u{i}"] = UP[e]
            m[f"d{i}"] = DP[e]
        in_maps.append(m)

    scatter_info = (T, dim, E, offs, pad_offs, counts, padded)
    return piece_sizes, plans, in_maps, scatter_info


def scatter(per_core_y, plans, scatter_info):
    """Assemble the full output from per-core y arrays."""
    T, dim, E, offs, pad_offs, counts, padded = scatter_info
    out_pad = np.empty((int(pad_offs[-1]), dim), dtype=np.float32)
    for c in range(NCORES):
        yc = np.asarray(per_core_y[c]).astype(np.float32)  # [NCH, 128, T_core]
        yc = yc.transpose(2, 0, 1).reshape(yc.shape[2], dim)  # [T_core, dim]
        t = 0
        for (e, bs, nb) in plans[c]:
            if bs >= 0:
                out_pad[bs * TB : (bs + nb) * TB] = yc[t : t + nb * TB]
            t += nb * TB
    if not padded:
        return out_pad
    out = np.empty((T, dim), dtype=np.float32)
    for e in range(E):
        out[offs[e] : offs[e + 1]] = out_pad[pad_offs[e] : pad_offs[e] + counts[e]]
    return out


def kernel(x, gate_proj, up_proj, down_proj, num_tokens_per_expert):
    piece_sizes, plans, in_maps, scatter_info = prepare(
        x, gate_proj, up_proj, down_proj, num_tokens_per_expert
    )
    dim = scatter_info[1]
    hid = np.asarray(gate_proj).shape[1]
    nc = build_program(piece_sizes, dim, hid)
    res = run_bass_kernel_spmd(nc, in_maps, core_ids=list(range(NCORES)))
    return scatter([res.results[c]["y"] for c in range(NCORES)], plans, scatter_info)
